# revision 3
# baseline (speedup 1.0000x reference)
"""Bass/Tile TRN2 kernel for nn_Attention_38276748542802 (Bahdanau-style
attention scores + masked softmax), data-parallel over 8 NeuronCores.

  h_part = hidden @ W[:256]                      # [B, 256]
  e_part = einsum('sbe,ed->sbd', enc, W[256:])   # [S, B, 256]
  energy = tanh(h_part + e_part + attn_b)
  scores = einsum('sbd,d->bs', energy, v); where(mask, -1e6); softmax over s

Shapes: B=128, S=1024, E=512, D=256.  Each core owns 16 batches.

Device-side layout: host supplies enc transposed to [E, B_loc*S] so the
contraction dim E lands on SBUF partitions (enc is e-contiguous in HBM,
while the PE contracts over the partition dim).  Matmuls run as float32r
(full-rate fp32 PE mode); tanh/softmax in fp32.
"""
import sys
sys.path.insert(0, '/opt/trn_rl_repo')
import numpy as np
import concourse.bass as bass
import concourse.bacc as bacc
import concourse.mybir as mybir
from concourse import tile
from concourse.bass_utils import run_bass_kernel_spmd

N_CORES = 8
B, S, E, D = 128, 1024, 512, 256
BL = B // N_CORES            # 16 batches per core
NCH = 2 * BL                 # 32 chunks of 512 rows (b, s-half)
F32 = mybir.dt.float32
F32R = mybir.dt.float32r
AFT = mybir.ActivationFunctionType
AX = mybir.AxisListType

_cache = {}


def _build():
    if "nc" in _cache:
        return _cache["nc"]
    nc = bacc.Bacc("TRN2", target_bir_lowering=False, debug=False, num_devices=1)
    d_enc = nc.dram_tensor("encT", [E, BL * S], F32R, kind="ExternalInput")
    d_we = nc.dram_tensor("w_e", [E, D], F32R, kind="ExternalInput")
    d_wh = nc.dram_tensor("w_h", [D, D], F32, kind="ExternalInput")
    d_hT = nc.dram_tensor("hiddenT", [D, BL], F32, kind="ExternalInput")
    d_ab = nc.dram_tensor("attn_b", [D, 1], F32, kind="ExternalInput")
    d_v = nc.dram_tensor("v", [D, 1], F32R, kind="ExternalInput")
    d_mask = nc.dram_tensor("maskadd", [BL, S], F32, kind="ExternalInput")
    d_out = nc.dram_tensor("out", [BL, S], F32, kind="ExternalOutput")

    with tile.TileContext(nc) as tc:
        with tc.tile_pool(name="const", bufs=1) as cp, \
             tc.tile_pool(name="io", bufs=3) as iop, \
             tc.tile_pool(name="work", bufs=3) as wp, \
             tc.tile_pool(name="pse", bufs=3, space="PSUM") as pse, \
             tc.tile_pool(name="pss", bufs=2, space="PSUM") as pss:

            # ---- enc chunk prefetch machinery ----
            enc3 = d_enc.ap().rearrange("(kt p) q -> p kt q", p=128)   # [128, 4, 16384]
            e_tiles = {}

            def load_chunk(c):
                b, sh = divmod(c, 2)
                col0 = b * S + sh * 512
                t = iop.tile([128, 4 * 512], F32R, name="e_sb")
                nc.sync.dma_start(
                    out=t.rearrange("p (kt j) -> p kt j", kt=4),
                    in_=enc3[:, :, col0:col0 + 512])
                e_tiles[c] = t

            # ---- constants (w_e first: first matmul needs it) ----
            w_e_sb = cp.tile([128, 4 * D], F32R)          # kt-major: [kt*256 + d]
            for kt in range(4):
                nc.sync.dma_start(out=w_e_sb[:, kt * D:(kt + 1) * D],
                                  in_=d_we.ap()[128 * kt:128 * (kt + 1), :])
            for c in range(PREFETCH):
                load_chunk(c)
            w_h_sb = cp.tile([128, 2 * D], F32)
            for kt in range(2):
                nc.sync.dma_start(out=w_h_sb[:, kt * D:(kt + 1) * D],
                                  in_=d_wh.ap()[128 * kt:128 * (kt + 1), :])
            hT_sb = cp.tile([128, 2 * BL], F32)
            for kt in range(2):
                nc.sync.dma_start(out=hT_sb[:, kt * BL:(kt + 1) * BL],
                                  in_=d_hT.ap()[128 * kt:128 * (kt + 1), :])
            ab_sb = cp.tile([128, 2], F32)
            v_sb = cp.tile([128, 2], F32R)
            for dt in range(2):
                nc.sync.dma_start(out=ab_sb[:, dt:dt + 1],
                                  in_=d_ab.ap()[128 * dt:128 * (dt + 1), :])
                nc.sync.dma_start(out=v_sb[:, dt:dt + 1],
                                  in_=d_v.ap()[128 * dt:128 * (dt + 1), :])

            # ---- h_part:  hb[d, b] = sum_k W_h[k, d] hiddenT[k, b] + attn_b[d]
            hb_sb = cp.tile([128, 2 * BL], F32)
            for dt in range(2):
                ph = pss.tile([128, BL], F32, name="ph")
                for kt in range(2):
                    nc.tensor.matmul(ph[:, :],
                                     w_h_sb[:, kt * D + dt * 128: kt * D + dt * 128 + 128],
                                     hT_sb[:, kt * BL:(kt + 1) * BL],
                                     start=(kt == 0), stop=(kt == 1))
                nc.scalar.activation(hb_sb[:, dt * BL:(dt + 1) * BL], ph[:, :],
                                     AFT.Identity, bias=ab_sb[:, dt:dt + 1], scale=1.0)

            # ---- main loop: 32 chunks of 512 rows (one b, one s-half each)
            scores_sb = cp.tile([1, BL * S], F32)
            scT = cp.tile([BL, S], F32)
            for c in range(NCH):
                b, sh = divmod(c, 2)
                if c + PREFETCH < NCH:
                    load_chunk(c + PREFETCH)
                e_sb = e_tiles.pop(c)
                ps_s = pss.tile([1, 512], F32, name="ps_s")
                for dt in range(2):
                    ps_e = pse.tile([128, 512], F32, name="ps_e")
                    for kt in range(4):
                        nc.tensor.matmul(
                            ps_e[:, :],
                            w_e_sb[:, kt * D + dt * 128: kt * D + dt * 128 + 128],
                            e_sb[:, kt * 512:(kt + 1) * 512],
                            start=(kt == 0), stop=(kt == 3))
                    t_sb = wp.tile([128, 512], F32R, name="t_sb")
                    nc.scalar.activation(t_sb[:, :], ps_e[:, :], AFT.Tanh,
                                         bias=hb_sb[:, dt * BL + b: dt * BL + b + 1],
                                         scale=1.0)
                    nc.tensor.matmul(ps_s[:, :], v_sb[:, dt:dt + 1], t_sb[:, :],
                                     start=(dt == 0), stop=(dt == 1))
                nc.vector.tensor_copy(scores_sb[:, c * 512:(c + 1) * 512], ps_s[:, :])
                if sh == 1:
                    # b's full row of scores is staged: scatter to partition b
                    nc.sync.dma_start(out=scT[b:b + 1, :],
                                      in_=scores_sb[:, b * S:(b + 1) * S])

            # ---- masked softmax over s, rows = b on partitions ----
            mask_sb = cp.tile([BL, S], F32)
            nc.sync.dma_start(out=mask_sb[:, :], in_=d_mask.ap())
            sc1 = cp.tile([BL, S], F32)
            nc.vector.tensor_add(sc1[:, :], scT[:, :], mask_sb[:, :])
            mx = cp.tile([BL, 1], F32)
            nc.vector.reduce_max(mx[:, :], sc1[:, :], axis=AX.X)
            nmx = cp.tile([BL, 1], F32)
            nc.vector.tensor_scalar_mul(nmx[:, :], mx[:, :], -1.0)
            ex = cp.tile([BL, S], F32)
            sm = cp.tile([BL, 1], F32)
            nc.scalar.activation(ex[:, :], sc1[:, :], AFT.Exp,
                                 bias=nmx[:, :], scale=1.0, accum_out=sm[:, :])
            rs = cp.tile([BL, 1], F32)
            nc.vector.reciprocal(rs[:, :], sm[:, :])
            outt = cp.tile([BL, S], F32)
            nc.vector.tensor_scalar_mul(outt[:, :], ex[:, :], rs[:, :])
            nc.sync.dma_start(out=d_out.ap(), in_=outt[:, :])

    nc.compile()
    _cache["nc"] = nc
    return nc


def make_in_maps(hidden, encoder_outputs, mask, attn_w, attn_b, v):
    hidden = np.asarray(hidden, dtype=np.float32)
    enc = np.asarray(encoder_outputs, dtype=np.float32)
    mask = np.asarray(mask)
    attn_w = np.asarray(attn_w, dtype=np.float32)
    attn_b = np.asarray(attn_b, dtype=np.float32)
    v = np.asarray(v, dtype=np.float32)

    w_h = np.ascontiguousarray(attn_w[:D])                      # [256, 256]
    w_e = np.ascontiguousarray(attn_w[D:])                      # [512, 256]
    ab = np.ascontiguousarray(attn_b.reshape(D, 1))
    vv = np.ascontiguousarray(v.reshape(D, 1))

    in_maps = []
    for m in range(N_CORES):
        bs = slice(BL * m, BL * (m + 1))
        encT = np.ascontiguousarray(
            enc[:, bs, :].transpose(2, 1, 0)).reshape(E, BL * S)
        hT = np.ascontiguousarray(hidden[bs].T)                 # [256, 16]
        maskadd = np.where(mask[bs], np.float32(-1e6),
                           np.float32(0.0)).astype(np.float32)
        in_maps.append({
            "encT": encT, "w_e": w_e, "w_h": w_h, "hiddenT": hT,
            "attn_b": ab, "v": vv, "maskadd": maskadd,
        })
    return in_maps


def kernel(hidden, encoder_outputs, mask, attn_w, attn_b, v):
    nc = _build()
    in_maps = make_in_maps(hidden, encoder_outputs, mask, attn_w, attn_b, v)
    res = run_bass_kernel_spmd(nc, in_maps, core_ids=list(range(N_CORES)))
    return np.concatenate([res.results[c]["out"] for c in range(N_CORES)],
                          axis=0).astype(np.float32)
